# revision 40
# baseline (speedup 1.0000x reference)
"""Trainium2 Bass kernel for BiLevelRoutingAttention (nn_BiLevelRoutingAttention_66907000537867).

Sharding: one attention head per NeuronCore (8 heads / 8 cores).

Single fused pipeline per core:
  phase 0: x half-tiles DMA'd up front across 3 queues (early tiles at
           finer granularity so the first qkv matmuls start ~2us in);
           weights; targeted border/pad memsets only.
  phase 1: qkv projection in bf16 over 7 window-row tiles. q (image
           order) + k (window order) evacuated to fp8e4 on ScalarE,
           v to the padded-image vs4 band split ScalarE/DVE; DVE stream
           transpose to pixel-major v with per-tile v_aug regroup DMAs.
           Band replication via SBUF-SBUF DMAs (fp8 halves the bytes).
  phase 2: per region (row-major): QK^T as 8 plain fp8e4 matmuls
           (contraction = 4 replicated partition bands, scale fused into
           exp as SCALE/4; DoubleRow measured slower: its 256-col
           ldweights cannot hide under 128-cycle matmuls) -> exp on
           ScalarE as 2x[128,1024] ACTIVATEs -> attn@V bf16 with a ones
           column producing softmax denominators -> per-window-row
           scatter/reciprocal/gather into a rotating recip row -> K=1
           broadcast matmul + DVE normalize. One junk matmul per region
           (NFILL) keeps the PE HAM clock-gate at 2.4 GHz; NWARM2 entry
           fillers bridge the kk-replication wait. Output-projection
           groups (phase 3) are interleaved one unit per region slot as
           soon as their window row is normalized; the drain loop after
           region 48 retires leftovers at 2 units/slot.
  phase 3 (interleaved): output projection with lepe folded in: vs4's
           four 32-partition bands hold three dy-shifted v copies + the
           normalized attention map; 3 dx-shift matmuls contract all
           128 partitions per group of 448 pixels. Output bf16; host
           sums the 8 per-head partials.

Host: region routing (top-k is metadata; the mean commutes with the linear
qkv layer), per-head weight slicing + lepe fold into dy-stacked
stationaries, bf16 casts, final partial-sum + constant bias row.
"""

import numpy as np
import ml_dtypes

import concourse.bass as bass
import concourse.bacc as bacc
import concourse.mybir as mybir
import concourse.tile as tile
from concourse.bass_utils import run_bass_kernel_spmd

F32 = mybir.dt.float32
BF16 = mybir.dt.bfloat16
FP8 = mybir.dt.float8e4
AF = mybir.ActivationFunctionType
DR = mybir.MatmulPerfMode.DoubleRow

DIM, QK, HEADS, NWIN, TOPK = 256, 256, 8, 7, 4
H = W = 112
P2 = NWIN * NWIN          # 49 regions
W2 = 256                  # pixels per region (16x16)
NPIX = H * W              # 12544
HD = 32                   # per-head dim
SCALE = QK ** (-0.5)      # 1/16
PW = 114                  # padded image width
TW = 1792                 # phase-1 tile = one window row (16 image rows)
NT = 7                    # phase-1 tiles
NG = 28                   # phase-3 pixel groups (4 image rows each)
N3 = 448                  # pixels per phase-3 group

USE_FP8_QKT = True        # fp8e4 q/k + DoubleRow QK^T (else bf16)
USE_DR = False            # A/B: DoubleRow vs plain fp8 (ld-hiding test)
NWARM1 = 8                # phase-1 PE clock-ramp junk matmuls
NWARM2 = 30               # phase-2 entry fillers (cover replication wait)

_cache = {}


def _build(top_idx, debug=False):
    nc = bacc.Bacc()
    xT_d = nc.declare_dram_parameter("xT", [DIM, NPIX], BF16, isOutput=False)
    wqkv_d = nc.declare_dram_parameter("wqkv", [DIM, 96], BF16, isOutput=False)
    bqkv_d = nc.declare_dram_parameter("bqkv", [96, 1], F32, isOutput=False)
    wt_d = nc.declare_dram_parameter("wt", [128, 768], BF16, isOutput=False)
    out_d = nc.declare_dram_parameter("out", [DIM, NPIX], BF16, isOutput=True)

    QKDT = FP8 if USE_FP8_QKT else BF16
    ESCALE = SCALE / 8 if (USE_FP8_QKT and USE_DR) else SCALE / 4

    chunk_list = [[2 * g + jj for g in top_idx[r] for jj in (0, 1)]
                  for r in range(P2)]
    # process regions in data-availability order: the latest phase-1 tile
    # touched by q block, k chunks, or the attn@V v_aug stationary span
    def need_tile(r):
        gmax = max(top_idx[r])
        return max(r // NWIN, gmax // NWIN, (2 * gmax + 4) // 14)
    region_order = list(range(P2))  # row-major: steady row completions

    with tile.TileContext(nc) as tc, tc.tile_pool(name="persist", bufs=1) as persist:
        # ---- persistent SBUF ----
        w_sb = persist.tile([128, 192], BF16)         # qkv weights, 2 cin chunks
        bqkv_sb = persist.tile([96, 1], F32)
        qq = persist.tile([128, NPIX], QKDT)          # q image-order, 4 bands
        kk = persist.tile([128, NPIX], QKDT)          # k image-order, 4 bands
        vs4 = persist.tile([128, PW * PW], BF16)
        v_aug = persist.tile([128, 102, 34], BF16)    # pixel-major v + ones col 32
        u_sb = persist.tile([33, NPIX], BF16)         # unnormalized attn out + denom
        out_att = persist.tile([32, NPIX], BF16)      # normalized attn (image order)
        dp_all = persist.tile([1, 3 * TW], BF16)      # rotating recip rows
        wt_sb = persist.tile([128, 768], BF16)        # dy-stacked proj taps
        ones_sb = persist.tile([1, 128], BF16)

        # ---- phase 0: weights + full x preload + targeted memsets ----
        nc.sync.dma_start(out=w_sb[:, 0:96], in_=wqkv_d[0:128, :])
        nc.sync.dma_start(out=w_sb[:, 96:192], in_=wqkv_d[128:256, :])
        nc.sync.dma_start(out=bqkv_sb, in_=bqkv_d[:, :])
        nc.sync.dma_start(out=wt_sb, in_=wt_d[:, :])

        vs4_v = vs4.rearrange("p (r c) -> p r c", c=PW)
        qq_v = qq.rearrange("p (r c) -> p r c", c=W)
        nc.vector.memset(ones_sb, 1.0)
        nc.vector.memset(v_aug[:, 98:102, :], 0.0)    # tail slots
        nc.vector.memset(v_aug[:, :, 32:33], 1.0)     # ones column
        nc.vector.memset(v_aug[:, 0:98, 33:34], 0.0)  # pad column
        nc.gpsimd.memset(vs4_v[64:96, 0:1, :], 0.0)       # v top border row
        nc.gpsimd.memset(vs4_v[64:96, 113:114, :], 0.0)   # v bottom border row
        nc.gpsimd.memset(vs4_v[64:96, :, 0:1], 0.0)       # v left border col
        nc.gpsimd.memset(vs4_v[64:96, :, 113:114], 0.0)   # v right border col
        nc.vector.memset(vs4_v[96:128, :, 112:114], 0.0)  # attn band right pad

        # ---- phase 1: qkv projection over 7 window-row tiles ----
        with (
            tc.tile_pool(name="xt", bufs=5) as xtp,
            tc.tile_pool(name="vtp", bufs=2) as vtp,
            tc.tile_pool(name="qkv_ps", bufs=2, space="PSUM") as qkvps,
        ):
            for t in range(NT):
                n0 = TW * t
                xt0 = xtp.tile([128, TW], BF16, tag="xt0", name="xt0")
                xt1 = xtp.tile([128, TW], BF16, tag="xt1", name="xt1")
                nc.sync.dma_start(out=xt0, in_=xT_d[0:128, n0:n0 + TW])
                nc.scalar.dma_start(out=xt1, in_=xT_d[128:256, n0:n0 + TW])
                ps = qkvps.tile([96, TW], F32, tag="qkv", name="ps")
                # clock-ramp junk on tile 0 (reads uninit u_sb: zero deps)
                for i in range(NWARM1 if t == 0 else 0):
                    nc.tensor.matmul(ps[:, 0:256], u_sb[0:33, 0:96],
                                     u_sb[0:33, 256 * i:256 * i + 256],
                                     start=True, stop=True)
                for blk in range(4):
                    c0, cw = 512 * blk, (512 if blk < 3 else 256)
                    nc.tensor.matmul(ps[:, c0:c0 + cw], w_sb[:, 0:96],
                                     xt0[:, c0:c0 + cw], start=True, stop=False)
                    nc.tensor.matmul(ps[:, c0:c0 + cw], w_sb[:, 96:192],
                                     xt1[:, c0:c0 + cw], start=False, stop=True)
                # q evac (image order) + k evac (window order) on ScalarE
                nc.scalar.activation(qq[0:32, n0:n0 + TW], ps[0:32, :],
                                     AF.Identity, bias=bqkv_sb[0:32, 0:1])
                kdst = kk[32:64, n0:n0 + TW].rearrange("p (w a b) -> p w a b",
                                                       a=16, b=16)
                ksrc = ps[32:64, :].rearrange("p (a w b) -> p w a b",
                                              w=NWIN, b=16)
                nc.scalar.activation(kdst, ksrc, AF.Identity,
                                     bias=bqkv_sb[32:64, 0:1])
                # v evac split ScalarE / DVE
                psv = ps[64:96, :].rearrange("p (a b) -> p a b", b=W)
                nc.scalar.activation(vs4_v[64:96, 16 * t + 1:16 * t + 5, 1:113],
                                     psv[:, 0:4, :], AF.Identity,
                                     bias=bqkv_sb[64:96, 0:1])
                nc.vector.tensor_scalar_add(
                    vs4_v[64:96, 16 * t + 5:16 * t + 17, 1:113],
                    psv[:, 4:16, :], bqkv_sb[64:96, 0:1])
                vsrc = vs4_v[64:96, 16 * t + 1:16 * t + 17, 1:113]
                vsrc = vsrc.rearrange("p a (w b) -> p w a b", b=16)
                vt = vtp.tile([128, TW], BF16, tag="vt", name="vt")
                nc.vector.transpose(vt[64:96, :], vsrc)
                # band replication (fp8 bytes)
                for b in (1, 2, 3):
                    nc.gpsimd.dma_start(out=qq[32 * b:32 * b + 32, n0:n0 + TW],
                                        in_=qq[0:32, n0:n0 + TW])
                for b in (0, 2, 3):
                    nc.sync.dma_start(out=kk[32 * b:32 * b + 32, n0:n0 + TW],
                                      in_=kk[32:64, n0:n0 + TW])
                # per-tile pixel-major v regroup
                vtt = vt.rearrange("p (c j) -> p c j", j=32)
                for a in range(4):
                    nc.gpsimd.dma_start(
                        out=v_aug[32 * a:32 * a + 32, 14 * t:14 * t + 14, 0:32],
                        in_=vtt[64:96, a:56:4, :])

        # ---- phases 2+3 fused: attention regions + interleaved out-proj ----
        with (
            tc.tile_pool(name="at_ps", bufs=1, space="PSUM") as atps,
            tc.tile_pool(name="av_ps", bufs=1, space="PSUM") as avps,
            tc.tile_pool(name="bc_ps", bufs=1, space="PSUM") as bcps,
            tc.tile_pool(name="o_ps", bufs=2, space="PSUM") as ops,
            tc.tile_pool(name="exp2", bufs=3) as expp,
            tc.tile_pool(name="dsc2", bufs=2) as dscp,
            tc.tile_pool(name="evsb", bufs=3) as evp,
        ):
            # dy-shifted vs4 bands (needed by first phase-3 group)
            nc.sync.dma_start(out=vs4[0:32, 0:PW * PW - PW],
                              in_=vs4[64:96, PW:PW * PW])
            nc.sync.dma_start(out=vs4[32:64, 0:PW * PW - 2 * PW],
                              in_=vs4[64:96, 2 * PW:PW * PW])

            st_prev = []           # [(region, chunks, ex)] awaiting attn@V
            norm_ready = []        # regions whose row reciprocals are queued
            p3q = []               # (group, half) out-proj units ready to run
            wr_av_left = [NWIN] * NWIN
            wr_norm_left = [NWIN] * NWIN

            def filler(dst_ps, n):
                # junk matmuls keep the PE HAM clock-gate at full speed
                for i in range(n):
                    nc.tensor.matmul(dst_ps, u_sb[0:33, 0:128],
                                     u_sb[0:33, 256 * i:256 * i + 256],
                                     start=True, stop=True)

            def emit_p3_unit(drain=False):
                # one unit = one group x one output half: 3 matmuls
                if not p3q:
                    return False
                g, hh = p3q.pop(0)
                ot = ops.tile([128, N3], F32, tag="p3", name="ot")
                for dx in range(3):
                    nc.tensor.matmul(
                        ot,
                        wt_sb[:, 128 * (2 * dx + hh):128 * (2 * dx + hh) + 128],
                        vs4_v[:, 4 * g:4 * g + 4, dx:dx + 112],
                        start=(dx == 0), stop=(dx == 2))
                ev = evp.tile([128, N3], BF16, tag="ev", name="ev")
                if drain:
                    # ScalarE is exp-free after the last region: use it
                    nc.scalar.copy(ev, ot)
                else:
                    nc.vector.tensor_copy(ev, ot)
                eng = nc.sync if (g + hh) % 2 == 0 else nc.gpsimd
                eng.dma_start(out=out_d[128 * hh:128 * hh + 128,
                                        N3 * g:N3 * g + N3], in_=ev)
                return True

            def emit_norm():
                # bc broadcast (K=1 matmul) + DVE normalize for one region
                if not norm_ready:
                    return False
                rn = norm_ready.pop(0)
                mwr, mwc = divmod(rn, NWIN)
                bc = bcps.tile([128, W2], F32, tag="bc", name="bc")
                d0 = TW * (mwr % 3)
                nc.tensor.matmul(bc, ones_sb[0:1, 0:128],
                                 dp_all[0:1, d0 + W2 * mwc:d0 + W2 * mwc + W2],
                                 start=True, stop=True)
                dst = out_att.rearrange("p (a c) -> p a c", c=W)
                dst = dst[0:32, 16 * mwr:16 * mwr + 16, 16 * mwc:16 * mwc + 16]
                uv = u_sb[0:32, W2 * rn:W2 * rn + W2]
                uv = uv.rearrange("p (a b) -> p a b", b=16)
                bcv = bc[0:32, :].rearrange("p (a b) -> p a b", b=16)
                nc.vector.tensor_mul(dst, uv, bcv)
                wr_norm_left[mwr] -= 1
                if wr_norm_left[mwr] == 0:
                    # completed row -> attention plane of vs4 (band 3)
                    nc.gpsimd.dma_start(
                        out=vs4_v[96:128, 16 * mwr:16 * mwr + 16, 0:112],
                        in_=out_att[0:32, TW * mwr:TW * mwr + TW])
                    p3q.extend((g, hh)
                               for g in range(4 * mwr, 4 * mwr + 4)
                               for hh in range(2))
                return True

            def emit_region_slot(r):
                # -- PE: QK^T_r
                if r is not None:
                    chunks = chunk_list[r]
                    wr, wc = divmod(r, NWIN)
                    atA = atps.tile([128, 1024], F32, tag="atA", name="atA")
                    atB = atps.tile([128, 1024], F32, tag="atB", name="atB")
                    qblk = qq_v[0:128, 16 * wr:16 * wr + 16,
                                16 * wc:16 * wc + 16]
                    if USE_FP8_QKT and USE_DR:
                        qdr = qblk.unsqueeze(1).broadcast_to([128, 2, 16, 16])
                    for j in range(8):
                        at = atA if j < 4 else atB
                        col = 256 * (j % 4)
                        c = chunks[j]
                        kblk = kk[0:128, 128 * c:128 * c + 128]
                        if USE_FP8_QKT and USE_DR:
                            kdr = kblk.unsqueeze(1).broadcast_to([128, 2, 128])
                            nc.tensor.matmul(at[:, col:col + 256], kdr, qdr,
                                             start=True, stop=True,
                                             perf_mode=DR)
                        else:
                            nc.tensor.matmul(at[:, col:col + 256], kblk,
                                             qblk, start=True, stop=True)
                # -- ScalarE: exp
                if r is not None:
                    ex = expp.tile([128, 2048], BF16, tag="ex", name="ex")
                    nc.scalar.activation(ex[:, 0:1024], atA, AF.Exp, scale=ESCALE)
                    nc.scalar.activation(ex[:, 1024:2048], atB, AF.Exp,
                                         scale=ESCALE)
                    st_prev.append((r, chunks, ex))
                # -- PE: attn@V of the previous region
                if len(st_prev) > (1 if r is not None else 0):
                    rp, chunks_p, ex_p = st_prev.pop(0)
                    avT = avps.tile([128, W2], F32, tag="av", name="avT")
                    vaf = v_aug.rearrange("p c j -> p (c j)")
                    for j in range(8):
                        nc.tensor.matmul(avT,
                                         vaf[:, 34 * chunks_p[j]:
                                             34 * chunks_p[j] + 128],
                                         ex_p[:, 256 * j:256 * j + 256],
                                         start=(j == 0), stop=(j == 7))
                    nc.vector.tensor_copy(u_sb[:, W2 * rp:W2 * rp + W2],
                                          avT[0:33, :])
                    mwr = rp // NWIN
                    wr_av_left[mwr] -= 1
                    if wr_av_left[mwr] == 0:
                        # row complete: invert its denominators on 128 lanes
                        n0 = TW * mwr
                        dsc = dscp.tile([128, 16], BF16, tag="dsc", name="dsc")
                        dsc2 = dscp.tile([128, 16], BF16, tag="dsc2", name="dsc2")
                        nc.gpsimd.dma_start(out=dsc[:, 0:14],
                                            in_=u_sb[32:33, n0:n0 + TW])
                        with nc.allow_low_precision(reason="bf16 softmax denom"):
                            nc.vector.reciprocal(dsc2[:, 0:14], dsc[:, 0:14])
                        d0 = TW * (mwr % 3)
                        nc.gpsimd.dma_start(out=dp_all[0:1, d0:d0 + TW],
                                            in_=dsc2[:, 0:14])
                        norm_ready.extend(range(NWIN * mwr, NWIN * mwr + NWIN))
                emit_norm()
                # -- interleaved out-projection work
                emit_p3_unit()

            filler(bcps.tile([128, W2], F32, tag="bc", name="bc"), NWARM2)
            for r in region_order:
                emit_region_slot(r)
            drain_iter = 0
            while st_prev or norm_ready or p3q:
                emit_region_slot(None)
                emit_norm()
                emit_p3_unit(drain=True)
                emit_p3_unit(drain=(drain_iter % 2 == 0))
                drain_iter += 1

    nc.compile()
    return nc


def _host_prep(x, w_qkv, b_qkv):
    xT = np.ascontiguousarray(
        x.reshape(NPIX, DIM).T).astype(ml_dtypes.bfloat16)
    xmean = x.reshape(NWIN, 16, NWIN, 16, DIM).mean((1, 3)).reshape(P2, DIM)
    q_win = xmean @ w_qkv[:, :QK] + b_qkv[:QK]
    k_win = xmean @ w_qkv[:, QK:2 * QK] + b_qkv[QK:2 * QK]
    logit = (q_win * SCALE) @ k_win.T
    top_idx = np.argsort(-logit, axis=-1, kind="stable")[:, :TOPK]
    return xT, top_idx


def _in_maps(x, w_qkv, b_qkv, w_o, lepe_w):
    xT, top_idx = _host_prep(x[0], w_qkv, b_qkv)
    lw = lepe_w[:, :, 0, :]  # [3,3,256]
    maps = []
    for h in range(HEADS):
        sl = slice(h * HD, (h + 1) * HD)
        wqkv_h = np.concatenate(
            [w_qkv[:, :QK][:, sl], w_qkv[:, QK:2 * QK][:, sl],
             w_qkv[:, 2 * QK:][:, sl]], axis=1).astype(ml_dtypes.bfloat16)
        bqkv_h = np.concatenate(
            [b_qkv[:QK][sl], b_qkv[QK:2 * QK][sl], b_qkv[2 * QK:][sl]])
        w_o_h = w_o[sl, :]  # [32, 256]
        # dy-stacked stationaries: per (dx, half), [128, 128]. Row blocks
        # match the vs4 band contents: p0-31 = dy1-shifted v, p32-63 = dy2,
        # p64-95 = dy0 (unshifted), p96-127 = attn (w_o, dx=0 only).
        row_of_dy = {1: 0, 2: 32, 0: 64}
        wt4 = np.zeros((128, 3, 2, 128), np.float32)
        for dx in range(3):
            for dy in range(3):
                blk = lw[dy, dx, sl][:, None] * w_o_h  # [32, 256]
                r0 = row_of_dy[dy]
                for hh in range(2):
                    wt4[r0:r0 + 32, dx, hh] = blk[:, 128 * hh:128 * hh + 128]
            if dx == 0:
                for hh in range(2):
                    wt4[96:128, 0, hh] = w_o_h[:, 128 * hh:128 * hh + 128]
        maps.append({
            "xT": xT,
            "wqkv": np.ascontiguousarray(wqkv_h),
            "bqkv": np.ascontiguousarray(bqkv_h[:, None]).astype(np.float32),
            "wt": np.ascontiguousarray(wt4.reshape(128, 768)).astype(
                ml_dtypes.bfloat16),
        })
    return maps, top_idx


def kernel(x, w_qkv, b_qkv, w_o, b_o, lepe_w, lepe_b):
    x = np.asarray(x, np.float32)
    w_qkv = np.asarray(w_qkv, np.float32)
    b_qkv = np.asarray(b_qkv, np.float32)
    w_o = np.asarray(w_o, np.float32)
    b_o = np.asarray(b_o, np.float32)
    lepe_w = np.asarray(lepe_w, np.float32)
    lepe_b = np.asarray(lepe_b, np.float32)

    maps, top_idx = _in_maps(x, w_qkv, b_qkv, w_o, lepe_w)
    key = top_idx.tobytes()
    if key not in _cache:
        _cache[key] = _build(top_idx)
    nc = _cache[key]

    res = run_bass_kernel_spmd(nc, maps, list(range(HEADS))).results
    total = np.zeros((DIM, NPIX), np.float32)
    for h in range(HEADS):
        total += np.asarray(res[h]["out"], np.float32)
    b_all = lepe_b @ w_o + b_o
    out = total.T + b_all
    return out.reshape(1, H, W, DIM).astype(np.float32)
